# revision 3
# baseline (speedup 1.0000x reference)
"""CrossAttention Trainium2 Bass kernel.

Problem: y = CrossAttention(x, kv) with the reference's no-transpose q-reshape
quirk, B=8, N=1024, C=768, H=8, D=96.

Strategy: pure data parallelism — batch element b on NeuronCore b. Host
pre-transposes x/kv/weights so every matmul contraction dim lands on SBUF
partitions. All matmuls run with fp16 operands (fp32 PSUM accumulate):
fp32r measures 2 cycles/row on HW for 512-wide moving dims while fp16/bf16
run at 1 cycle/row, and fp16 keeps rel err ~7e-4 (gate is 2e-2).

Per-core pipeline (all on-chip after the input DMAs):
  P2  qp^T = Wq^T.T @ x^T        -> QT[d, h*1024+n] (the reshape quirk makes
      head h's Q^T a contiguous slice of qp rows; handled by a strided copy)
  P3  K half of kv proj          -> KT[d, h*1024+n]
  P4  V half, natural layout     -> V[nb][k, 97h+d], col 97h+96 = 1.0 (ones
      column makes the PV matmul also produce the softmax row-sums)
  P5  per head: S^T = KT.T @ QT -> exp (no max-subtract; |S|<~6 so exp is
      safe) -> P~^T; O~aug^T = V.T @ P~^T (row 96 = rowsum). S(h+1) is
      emitted before PV(h) so the ACT exp pipeline never starves. Each head's
      normalization chain (rowsum -> DMA repartition -> recip -> DMA
      broadcast -> in-place multiply) is emitted right after its PV so it
      overlaps later heads' attention.
  P7  y = sum_h O_h^T.T @ Wproj^T_h (+ bias via the ones row of head 7,
      which the normalization turned into exactly 1.0)
"""
import sys
sys.path.insert(0, '/opt/trn_rl_repo')

import numpy as np
import concourse.bass as bass
import concourse.mybir as mybir
import concourse.tile as tile
from concourse.bass_utils import run_bass_kernel_spmd

F32 = mybir.dt.float32
F16 = mybir.dt.float16
AF = mybir.ActivationFunctionType

B, N, C = 8, 1024, 768
H, D = 8, 96
SCALE = D ** -0.5
NB = N // 128   # 8 n-blocks
CB = C // 128   # 6 c-blocks
HN = H * N      # 8192


def _legalize_waits(nc, max_waits=1):
    """This container's walrus accepts at most one sync-wait command per
    instruction; move excess waits onto preceding NoOps on the same engine."""
    ctr = 0
    for f in nc.m.functions:
        for blk in f.blocks:
            out = []
            changed = False
            for ins in blk.instructions:
                si = ins.sync_info
                waits = list(si.on_wait) if si is not None and si.on_wait else []
                if len(waits) > max_waits:
                    changed = True
                    for w in waits[:-max_waits]:
                        ctr += 1
                        nop = mybir.InstNoOp(name=f"I-wsplit-{ctr}")
                        nop.engine = ins.engine
                        nop.sync_info = mybir.SyncInfo(on_wait=[w], on_update=[])
                        out.append(nop)
                    ins.sync_info = mybir.SyncInfo(
                        on_wait=waits[-max_waits:],
                        on_update=list(si.on_update or []))
                out.append(ins)
            if changed:
                blk.instructions = out
    return ctr


def build_kernel(repeat=1):
    nc = bass.Bass('TRN2', target_bir_lowering=False, debug=False, num_devices=B)

    xT = nc.dram_tensor("xT", [C, N], F16, kind="ExternalInput").ap()
    kvT = nc.dram_tensor("kvT", [C, N], F16, kind="ExternalInput").ap()
    WqT = nc.dram_tensor("WqT", [C, C], F16, kind="ExternalInput").ap()
    WkvT = nc.dram_tensor("WkvT", [C, 2 * C], F16, kind="ExternalInput").ap()
    WpjT = nc.dram_tensor("WpjT", [C, C], F16, kind="ExternalInput").ap()
    bias = nc.dram_tensor("bias", [1, C], F16, kind="ExternalInput").ap()
    y = nc.dram_tensor("y", [N, C], F16, kind="ExternalOutput").ap()
    rs_dram = nc.dram_tensor("rs_scratch", [1, HN], F16, kind="Internal").ap()
    ri_dram = nc.dram_tensor("ri_scratch", [1, HN], F16, kind="Internal").ap()

    with tile.TileContext(nc) as tc:
      for _rep in range(repeat):
        with tc.tile_pool(name="persist", bufs=1) as pp, \
             tc.tile_pool(name="norm", bufs=1) as pn, \
             tc.tile_pool(name="pta", bufs=1) as ppa, \
             tc.tile_pool(name="psum_mm", bufs=2, space="PSUM") as pmm, \
             tc.tile_pool(name="psum_o", bufs=4, space="PSUM") as pso:
            QT = pp.tile([D, HN], F16, tag="QT")
            KT = pp.tile([D, HN], F16, tag="KT")
            V = [pp.tile([128, H * 97], F16, tag=f"V{i}", name=f"V{i}")
                 for i in range(NB)]

            with tc.tile_pool(name="wkv", bufs=1) as pwkv:
                kvTs = [pwkv.tile([128, N], F16, tag=f"kv{i}", name=f"kvTs{i}")
                        for i in range(CB)]
                WkvTs = [pwkv.tile([128, 2 * C], F16, tag=f"Wkv{i}",
                                   name=f"WkvTs{i}") for i in range(CB)]
                with tc.tile_pool(name="wq", bufs=1) as pwq:
                    xTs = [pwq.tile([128, N], F16, tag=f"xT{i}",
                                    name=f"xTs{i}") for i in range(CB)]
                    WqTs = [pwq.tile([128, C], F16, tag=f"Wq{i}",
                                     name=f"WqTs{i}") for i in range(CB)]
                    for i in range(CB):
                        nc.sync.dma_start(WqTs[i][:],
                                          WqT[128 * i:128 * (i + 1), :])
                        nc.sync.dma_start(xTs[i][:, 0:512],
                                          xT[128 * i:128 * (i + 1), 0:512])
                    for i in range(CB):
                        nc.sync.dma_start(xTs[i][:, 512:1024],
                                          xT[128 * i:128 * (i + 1), 512:1024])
                    for i in range(CB):
                        nc.sync.dma_start(WkvTs[i][:],
                                          WkvT[128 * i:128 * (i + 1), :])
                        nc.sync.dma_start(kvTs[i][:],
                                          kvT[128 * i:128 * (i + 1), :])

                    ones_stage = pp.tile([128, 8], F32, tag="ones")
                    nc.vector.memset(ones_stage[:], 1.0)
                    for nb in range(NB):
                        ones_cols = V[nb][:].rearrange(
                            "p (h c) -> p h c", h=H)[:, :, 96:97]
                        nc.vector.tensor_copy(ones_cols, ones_stage[:])

                    # P2: Q projection -> QT (strided dest: reshape quirk)
                    # Wave-structured (cb outer) so PE consumes input tiles
                    # as the DMAs deliver them: 8 concurrent psum groups.
                    _wv = [0]

                    def proj_wave(groups, lhsT_of, rhs_of, evac, mm_parts=D):
                        for i in range(0, len(groups), 8):
                            wave = groups[i:i + 8]
                            _wv[0] += 1
                            ts = [pmm.tile([128, 1024], F32, tag="mm",
                                           name=f"wmm{_wv[0]}_{j}")
                                  for j in range(2)]
                            slots = [ts[0][0:mm_parts, 0:512],
                                     ts[0][0:mm_parts, 512:1024],
                                     ts[1][0:mm_parts, 0:512],
                                     ts[1][0:mm_parts, 512:1024]] + [
                                pso.tile([128, 512], F32, tag="po",
                                         name=f"wpo{_wv[0]}_{j}")
                                [0:mm_parts, 0:512] for j in range(4)]
                            for cb in range(CB):
                                for g, ps in zip(wave, slots):
                                    nc.tensor.matmul(
                                        ps, lhsT_of(g, cb), rhs_of(g, cb),
                                        start=(cb == 0), stop=(cb == CB - 1))
                            for g, ps in zip(wave, slots):
                                evac(g, ps)

                    def q_evac(g, ps):
                        r, u = g
                        dest = QT[:].rearrange(
                            "p (h j r) -> p h j r", h=H, j=128)[
                            :, 4 * u:4 * (u + 1), :, r:r + 1]
                        nc.vector.tensor_copy(dest, ps)

                    # Wave 1 (u=0) streams against the arriving x/Wq DMAs;
                    # everything later runs group-serial (inputs resident, so
                    # each group's evac overlaps the next group's matmuls —
                    # no wave-boundary WAR stall on the psum slots).
                    _sv = [0]

                    def proj_serial(groups, lhsT_of, rhs_of, evac,
                                    mm_parts, ncols):
                        for g in groups:
                            _sv[0] += 1
                            ps = pso.tile([128, 512], F32, tag="po",
                                          name=f"ser{_sv[0]}")
                            ps = ps[0:mm_parts, 0:ncols]
                            for cb in range(CB):
                                nc.tensor.matmul(
                                    ps, lhsT_of(g, cb), rhs_of(g, cb),
                                    start=(cb == 0), stop=(cb == CB - 1))
                            evac(g, ps)

                    q_lhsT = lambda g, cb: WqTs[cb][:, 96 * g[0]:96 * (g[0] + 1)]
                    q_rhs = lambda g, cb: xTs[cb][:, 512 * g[1]:512 * (g[1] + 1)]
                    proj_wave([(r, 0) for r in range(8)], q_lhsT, q_rhs,
                              q_evac)
                    proj_serial([(r, 1) for r in range(8)], q_lhsT, q_rhs,
                                q_evac, D, 512)

                # P3: K projection -> KT (group-serial)
                def k_evac(g, ps):
                    h, u = g
                    nc.vector.tensor_copy(
                        KT[:, 1024 * h + 512 * u:
                           1024 * h + 512 * (u + 1)], ps)

                proj_serial(
                    [(h, u) for h in range(H) for u in range(2)],
                    lambda g, cb: WkvTs[cb][:, 96 * g[0]:96 * (g[0] + 1)],
                    lambda g, cb: kvTs[cb][:, 512 * g[1]:512 * (g[1] + 1)],
                    k_evac, D, 512)

                # Early first S tile + exp: warms the ACT table and
                # starts the exp pipeline during the projection phase.
                P0_first = ppa.tile([128, N], F16, tag="pta", name="P0f")
                ps0 = pmm.tile([128, 1024], F32, tag="mm", name="s0_early")
                for u in range(2):
                    nc.tensor.matmul(
                        ps0[:, 512 * u:512 * (u + 1)],
                        KT[:, 0:128],
                        QT[:, 512 * u:512 * (u + 1)],
                        start=True, stop=True)
                nc.scalar.activation(P0_first[:], ps0[:], AF.Exp)

                # P4: V projection, natural layout + ones columns
                def v_evac(g, ps):
                    nb, u = g
                    dest = V[nb][:].rearrange(
                        "p (h c) -> p h c", h=H)[
                        :, 4 * u:4 * (u + 1), 0:96]
                    nc.vector.tensor_copy(dest, ps)

                kv_wave(
                    [(nb, u) for nb in range(NB) for u in range(2)],
                    lambda g, cb: kvTs[cb][:, 128 * g[0]:128 * (g[0] + 1)],
                    lambda g, cb: WkvTs[cb][:, C + 384 * g[1]:
                                            C + 384 * (g[1] + 1)],
                    v_evac, 128, 384)

            with tc.tile_pool(name="oa", bufs=1) as poa:
                Oall = poa.tile([97, HN], F16, tag="Oall")
                # Wproj tiles load during P5 (the pool opens after wkv/wq free)
                Wp = []
                for h in range(H):
                    rows = 97 if h == H - 1 else 96
                    t = poa.tile([rows, C], F16, tag=f"Wp{h}", name=f"Wp{h}")
                    nc.sync.dma_start(t[0:96, :], WpjT[96 * h:96 * (h + 1), :])
                    Wp.append(t)
                nc.sync.dma_start(Wp[H - 1][96:97, :], bias[:])

                with tc.tile_pool(name="pt", bufs=10) as ppt:
                    def emit_S(h, P_of, kb_start=0):
                        for kb in range(kb_start, NB):
                            ps = pmm.tile([128, 1024], F32, tag="mm",
                                          name=f"s{h}_{kb}")
                            for u in range(2):
                                nc.tensor.matmul(
                                    ps[:, 512 * u:512 * (u + 1)],
                                    KT[:, 1024 * h + 128 * kb:
                                       1024 * h + 128 * (kb + 1)],
                                    QT[:, 1024 * h + 512 * u:
                                       1024 * h + 512 * (u + 1)],
                                    start=True, stop=True)
                            nc.scalar.activation(P_of[kb][:], ps[:], AF.Exp)

                    def emit_PV(h, P_of):
                        for u in range(2):
                            po = pso.tile([97, 512], F32, tag="po",
                                          name=f"po{h}_{u}")
                            for kb in range(NB):
                                nc.tensor.matmul(
                                    po[:], V[kb][:, 97 * h:97 * (h + 1)],
                                    P_of[kb][:, 512 * u:512 * (u + 1)],
                                    start=(kb == 0), stop=(kb == NB - 1))
                            nc.vector.tensor_copy(
                                Oall[:, 1024 * h + 512 * u:
                                     1024 * h + 512 * (u + 1)], po[:])

                    ones97f = pn.tile([1, 97], F32, tag="o97f")
                    nc.vector.memset(ones97f[:], 1.0)
                    ones97 = pn.tile([1, 97], F16, tag="o97")
                    nc.vector.tensor_copy(ones97[:], ones97f[:])

                    def emit_norm(h):
                        """rowsum -> 1/rowsum broadcast to all partitions ->
                        in-place normalize Oall's head-h slice."""
                        sl = slice(1024 * h, 1024 * (h + 1))
                        nc.sync.dma_start(rs_dram[0:1, sl], Oall[96:97, sl])
                        rsh = pn.tile([128, 8], F16, tag="rs", name=f"rs{h}", bufs=2)
                        nc.sync.dma_start(
                            rsh[:],
                            rs_dram[0:1, sl].rearrange(
                                "p (a b) -> (p a) b", a=128))
                        rih = pn.tile([128, 8], F32, tag="ri", name=f"ri{h}", bufs=2)
                        nc.vector.reciprocal(rih[:], rsh[:])
                        rirh = pn.tile([128, 8], F16, tag="rir",
                                       name=f"rir{h}", bufs=2)
                        nc.vector.tensor_copy(rirh[:], rih[:])
                        nc.sync.dma_start(
                            ri_dram[0:1, sl].rearrange(
                                "p (a b) -> (p a) b", a=128), rirh[:])
                        bch = pn.tile([97, N], F16, tag="bc", name=f"bc{h}")
                        nc.sync.dma_start(
                            bch[:], bass.AP(ri_dram.tensor, 1024 * h,
                                            [[0, 97], [1, N]]))
                        nc.vector.tensor_mul(Oall[:, sl], Oall[:, sl], bch[:])

                    def emit_norm_fast(h):
                        """Tail variant with no DMAs on the critical path:
                        inv = exp(-ln(rowsum)) on ACT (same table set as the
                        softmax Exp), broadcast via a K=1 ones matmul on PE,
                        multiply from PSUM."""
                        sl = slice(1024 * h, 1024 * (h + 1))
                        lnr = pn.tile([1, N], F32, tag="lnx", name=f"lnr{h}")
                        nc.scalar.activation(lnr[:], Oall[96:97, sl], AF.Ln)
                        invt = pn.tile([1, N], F16, tag="invr",
                                       name=f"invr{h}")
                        nc.scalar.activation(invt[:], lnr[:], AF.Exp,
                                             scale=-1.0)
                        invr = invt[:]
                        for u in range(2):
                            bc_ps = pso.tile([97, 512], F32, tag="po",
                                             name=f"bcps{h}_{u}")
                            nc.tensor.matmul(
                                bc_ps[:], ones97[:],
                                invr[0:1, 512 * u:512 * (u + 1)],
                                start=True, stop=True)
                            ssl = slice(1024 * h + 512 * u,
                                        1024 * h + 512 * (u + 1))
                            nc.vector.tensor_mul(Oall[:, ssl], Oall[:, ssl],
                                                 bc_ps[:])

                    def emit_yproj(nb, heads, first, last, py):
                        """Partial output projection over `heads` for n-block
                        nb. first: start accumulation DMA (bypass write);
                        last: DMA-accumulate into y."""
                        ysb = py.tile([128, C], F16, tag="ysb",
                                      name=f"ysb{nb}_{heads[0]}")
                        for u in range(2):
                            ps = pmm.tile([128, 384], F32, tag="mm")
                            for i, h in enumerate(heads):
                                rows = 97 if h == H - 1 else 96
                                nc.tensor.matmul(
                                    ps[:],
                                    Oall[0:rows, 1024 * h + 128 * nb:
                                         1024 * h + 128 * (nb + 1)],
                                    Wp[h][0:rows, 384 * u:384 * (u + 1)],
                                    start=(i == 0), stop=(i == len(heads) - 1))
                            nc.vector.tensor_copy(
                                ysb[:, 384 * u:384 * (u + 1)], ps[:])
                        if first:
                            nc.sync.dma_start(
                                y[128 * nb:128 * (nb + 1), :], ysb[:])
                        else:
                            nc.gpsimd.dma_start(
                                y[128 * nb:128 * (nb + 1), :], ysb[:],
                                accum_op=mybir.AluOpType.add)

                    with tc.tile_pool(name="yout", bufs=2) as py:
                        P_tiles = {}
                        P_tiles[0] = [P0_first] + [
                            ppt.tile([128, N], F16, tag="pt", name=f"P0_{i}")
                            for i in range(1, NB)]
                        emit_S(0, P_tiles[0], kb_start=1)
                        for h in range(H):
                            if h + 1 < H:
                                P_tiles[h + 1] = [
                                    ppt.tile([128, N], F16, tag="pt",
                                             name=f"P{h + 1}_{i}")
                                    for i in range(NB)]
                                emit_S(h + 1, P_tiles[h + 1])
                            emit_PV(h, P_tiles.pop(h))
                            if h >= 6:
                                emit_norm_fast(h)
                            else:
                                emit_norm(h)
                        for nb in range(NB):
                            emit_yproj(nb, [0, 1, 2, 3, 4, 5, 6, 7],
                                       True, True, py)

    _legalize_waits(nc)
    return nc


def prep_in_maps(x, kv, Wq, Wkv, Wproj, bproj):
    """Host-side prep: transpose + fp16 cast, one in_map per core/batch."""
    x = np.asarray(x, dtype=np.float32)
    kv = np.asarray(kv, dtype=np.float32)
    WqTs = (np.ascontiguousarray(np.asarray(Wq, np.float32).T)
            * np.float32(SCALE)).astype(np.float16)
    WkvT = np.ascontiguousarray(np.asarray(Wkv, np.float32).T).astype(np.float16)
    WpjT = np.ascontiguousarray(np.asarray(Wproj, np.float32).T).astype(np.float16)
    bias_np = np.asarray(bproj, np.float32).reshape(1, C).astype(np.float16)
    in_maps = []
    for b in range(B):
        in_maps.append({
            "xT": np.ascontiguousarray(x[b].T).astype(np.float16),
            "kvT": np.ascontiguousarray(kv[b].T).astype(np.float16),
            "WqT": WqTs,
            "WkvT": WkvT,
            "WpjT": WpjT,
            "bias": bias_np,
        })
    return in_maps


_NC_CACHE = {}


def kernel(x, kv, Wq, Wkv, Wproj, bproj, _trace=False):
    in_maps = prep_in_maps(x, kv, Wq, Wkv, Wproj, bproj)
    if "nc" not in _NC_CACHE:
        _NC_CACHE["nc"] = build_kernel()
    nc = _NC_CACHE["nc"]
    res = run_bass_kernel_spmd(nc, in_maps, core_ids=list(range(B)),
                               trace=_trace)
    out = np.stack([r["y"] for r in res.results]).astype(np.float32)
    if _trace:
        return out, res
    return out


# revision 8
# speedup vs baseline: 1.3512x; 1.3512x over previous
"""CrossAttention Trainium2 Bass kernel.

Problem: y = CrossAttention(x, kv) with the reference's no-transpose q-reshape
quirk, B=8, N=1024, C=768, H=8, D=96.

Strategy: pure data parallelism — batch element b on NeuronCore b. Host
pre-transposes x/kv/weights so every matmul contraction dim lands on SBUF
partitions. All matmuls run with fp16 operands (fp32 PSUM accumulate):
fp32r measures 2 cycles/row on HW for 512-wide moving dims while fp16/bf16
run at 1 cycle/row, and fp16 keeps rel err ~7e-4 (gate is 2e-2).

Per-core pipeline (all on-chip after the input DMAs):
  P2  qp^T = Wq^T.T @ x^T        -> QT[d, h*1024+n] (the reshape quirk makes
      head h's Q^T a contiguous slice of qp rows; handled by a strided copy)
  P3  K half of kv proj          -> KT[d, h*1024+n]
  P4  V half, natural layout     -> V[nb][k, 97h+d], col 97h+96 = 1.0 (ones
      column makes the PV matmul also produce the softmax row-sums)
  P5  per head: S^T = KT.T @ QT -> exp (no max-subtract; |S|<~6 so exp is
      safe) -> P~^T; O~aug^T = V.T @ P~^T (row 96 = rowsum). S(h+1) is
      emitted before PV(h) so the ACT exp pipeline never starves. Each head's
      normalization chain (rowsum -> DMA repartition -> recip -> DMA
      broadcast -> in-place multiply) is emitted right after its PV so it
      overlaps later heads' attention.
  P7  y = sum_h O_h^T.T @ Wproj^T_h (+ bias via the ones row of head 7,
      which the normalization turned into exactly 1.0)
"""
import sys
sys.path.insert(0, '/opt/trn_rl_repo')

import numpy as np
import concourse.bass as bass
import concourse.mybir as mybir
import concourse.tile as tile
from concourse.bass_utils import run_bass_kernel_spmd

F32 = mybir.dt.float32
F16 = mybir.dt.float16
BF16 = mybir.dt.bfloat16
import os as _os
if _os.environ.get("KERNEL_DT16", "f16") == "bf16":
    import ml_dtypes as _mld
    DT16, NP16 = BF16, _mld.bfloat16
else:
    DT16, NP16 = F16, np.float16
AF = mybir.ActivationFunctionType

B, N, C = 8, 1024, 768
H, D = 8, 96
SCALE = D ** -0.5
NB = N // 128   # 8 n-blocks
CB = C // 128   # 6 c-blocks
HN = H * N      # 8192


def _legalize_waits(nc, max_waits=1):
    """This container's walrus accepts at most one sync-wait command per
    instruction; move excess waits onto preceding NoOps on the same engine."""
    ctr = 0
    for f in nc.m.functions:
        for blk in f.blocks:
            out = []
            changed = False
            for ins in blk.instructions:
                si = ins.sync_info
                waits = list(si.on_wait) if si is not None and si.on_wait else []
                if len(waits) > max_waits:
                    changed = True
                    for w in waits[:-max_waits]:
                        ctr += 1
                        nop = mybir.InstNoOp(name=f"I-wsplit-{ctr}")
                        nop.engine = ins.engine
                        nop.sync_info = mybir.SyncInfo(on_wait=[w], on_update=[])
                        out.append(nop)
                    ins.sync_info = mybir.SyncInfo(
                        on_wait=waits[-max_waits:],
                        on_update=list(si.on_update or []))
                out.append(ins)
            if changed:
                blk.instructions = out
    return ctr


def build_kernel(repeat=1):
    nc = bass.Bass('TRN2', target_bir_lowering=False, debug=False, num_devices=B)

    xT = nc.dram_tensor("xT", [C, N], DT16, kind="ExternalInput").ap()
    kvT = nc.dram_tensor("kvT", [C, N], DT16, kind="ExternalInput").ap()
    WqT = nc.dram_tensor("WqT", [C, C], DT16, kind="ExternalInput").ap()
    WkvT = nc.dram_tensor("WkvT", [C, 2 * C], DT16, kind="ExternalInput").ap()
    WpjT = nc.dram_tensor("WpjT", [C, C], DT16, kind="ExternalInput").ap()
    bias = nc.dram_tensor("bias", [1, C], DT16, kind="ExternalInput").ap()
    y = nc.dram_tensor("y", [N, C], DT16, kind="ExternalOutput").ap()
    rs_dram = nc.dram_tensor("rs_scratch", [1, HN], DT16, kind="Internal").ap()
    ri_dram = nc.dram_tensor("ri_scratch", [1, HN], DT16, kind="Internal").ap()

    with tile.TileContext(nc) as tc:
      for _rep in range(repeat):
        with tc.tile_pool(name="persist", bufs=1) as pp, \
             tc.tile_pool(name="norm", bufs=1) as pn, \
             tc.tile_pool(name="pta", bufs=1) as ppa, \
             tc.tile_pool(name="psum_mm", bufs=2, space="PSUM") as pmm, \
             tc.tile_pool(name="psum_o", bufs=4, space="PSUM") as pso:
            QT = pp.tile([D, HN], DT16, tag="QT")
            KT = pp.tile([D, HN], DT16, tag="KT")
            V = [pp.tile([128, H * 97], DT16, tag=f"V{i}", name=f"V{i}")
                 for i in range(NB)]

            with tc.tile_pool(name="wkv", bufs=1) as pwkv:
                kvTs = [pwkv.tile([128, N], DT16, tag=f"kv{i}", name=f"kvTs{i}")
                        for i in range(CB)]
                WkvTs = [pwkv.tile([128, 2 * C], DT16, tag=f"Wkv{i}",
                                   name=f"WkvTs{i}") for i in range(CB)]
                with tc.tile_pool(name="wq", bufs=1) as pwq:
                    xTs = [pwq.tile([128, N], DT16, tag=f"xT{i}",
                                    name=f"xTs{i}") for i in range(CB)]
                    WqTs = [pwq.tile([128, C], DT16, tag=f"Wq{i}",
                                     name=f"WqTs{i}") for i in range(CB)]
                    for i in range(CB):
                        nc.sync.dma_start(WqTs[i][:],
                                          WqT[128 * i:128 * (i + 1), :])
                        nc.sync.dma_start(xTs[i][:, 0:512],
                                          xT[128 * i:128 * (i + 1), 0:512])
                    for i in range(CB):
                        nc.sync.dma_start(xTs[i][:, 512:1024],
                                          xT[128 * i:128 * (i + 1), 512:1024])
                    for i in range(CB):
                        nc.sync.dma_start(WkvTs[i][:],
                                          WkvT[128 * i:128 * (i + 1), :])
                        nc.sync.dma_start(kvTs[i][:],
                                          kvT[128 * i:128 * (i + 1), :])

                    ones_stage = pp.tile([128, 8], F32, tag="ones")
                    nc.vector.memset(ones_stage[:], 1.0)
                    for nb in range(NB):
                        ones_cols = V[nb][:].rearrange(
                            "p (h c) -> p h c", h=H)[:, :, 96:97]
                        nc.vector.tensor_copy(ones_cols, ones_stage[:])

                    # P2: Q projection -> QT (strided dest: reshape quirk)
                    # Wave-structured (cb outer) so PE consumes input tiles
                    # as the DMAs deliver them: 8 concurrent psum groups.
                    _wv = [0]

                    def proj_wave(groups, lhsT_of, rhs_of, evac, mm_parts=D):
                        for i in range(0, len(groups), 8):
                            wave = groups[i:i + 8]
                            _wv[0] += 1
                            ts = [pmm.tile([128, 1024], F32, tag="mm",
                                           name=f"wmm{_wv[0]}_{j}")
                                  for j in range(2)]
                            slots = [ts[0][0:mm_parts, 0:512],
                                     ts[0][0:mm_parts, 512:1024],
                                     ts[1][0:mm_parts, 0:512],
                                     ts[1][0:mm_parts, 512:1024]] + [
                                pso.tile([128, 512], F32, tag="po",
                                         name=f"wpo{_wv[0]}_{j}")
                                [0:mm_parts, 0:512] for j in range(4)]
                            for cb in range(CB):
                                for g, ps in zip(wave, slots):
                                    nc.tensor.matmul(
                                        ps, lhsT_of(g, cb), rhs_of(g, cb),
                                        start=(cb == 0), stop=(cb == CB - 1))
                            # pso slots evacuate first: the serial phase that
                            # follows allocates from the same ring.
                            order = list(zip(wave, slots))
                            for g, ps in order[4:] + order[:4]:
                                evac(g, ps)

                    def q_evac(g, ps):
                        r, u = g
                        dest = QT[:].rearrange(
                            "p (h j r) -> p h j r", h=H, j=128)[
                            :, 4 * u:4 * (u + 1), :, r:r + 1]
                        nc.vector.tensor_copy(dest, ps)

                    # Wave 1 (u=0) streams against the arriving x/Wq DMAs;
                    # everything later runs group-serial (inputs resident, so
                    # each group's evac overlaps the next group's matmuls —
                    # no wave-boundary WAR stall on the psum slots).
                    _sv = [0]

                    def proj_serial(groups, lhsT_of, rhs_of, evac,
                                    mm_parts, ncols):
                        for g in groups:
                            _sv[0] += 1
                            ps = pso.tile([128, 512], F32, tag="po",
                                          name=f"ser{_sv[0]}")
                            ps = ps[0:mm_parts, 0:ncols]
                            for cb in range(CB):
                                nc.tensor.matmul(
                                    ps, lhsT_of(g, cb), rhs_of(g, cb),
                                    start=(cb == 0), stop=(cb == CB - 1))
                            evac(g, ps)

                    q_lhsT = lambda g, cb: WqTs[cb][:, 96 * g[0]:96 * (g[0] + 1)]
                    q_rhs = lambda g, cb: xTs[cb][:, 512 * g[1]:512 * (g[1] + 1)]
                    proj_wave([(r, 0) for r in range(8)], q_lhsT, q_rhs,
                              q_evac)
                    proj_serial([(r, 1) for r in range(8)], q_lhsT, q_rhs,
                                q_evac, D, 512)

                # P3: K projection -> KT (group-serial)
                def k_evac(g, ps):
                    h, u = g
                    nc.vector.tensor_copy(
                        KT[:, 1024 * h + 512 * u:
                           1024 * h + 512 * (u + 1)], ps)

                proj_serial(
                    [(h, u) for h in range(H) for u in range(2)],
                    lambda g, cb: WkvTs[cb][:, 96 * g[0]:96 * (g[0] + 1)],
                    lambda g, cb: kvTs[cb][:, 512 * g[1]:512 * (g[1] + 1)],
                    k_evac, D, 512)

                # Early first S tile + exp: warms the ACT table and
                # starts the exp pipeline during the projection phase.
                P0_first = ppa.tile([128, N], DT16, tag="pta", name="P0f")
                ps0 = pmm.tile([128, 1024], F32, tag="mm", name="s0_early")
                for u in range(2):
                    nc.tensor.matmul(
                        ps0[:, 512 * u:512 * (u + 1)],
                        KT[:, 0:128],
                        QT[:, 512 * u:512 * (u + 1)],
                        start=True, stop=True)
                nc.scalar.activation(P0_first[:], ps0[:], AF.Exp)

                # P4: V projection, natural layout + ones columns
                def v_evac(g, ps):
                    nb, u = g
                    dest = V[nb][:].rearrange(
                        "p (h c) -> p h c", h=H)[
                        :, 4 * u:4 * (u + 1), 0:96]
                    nc.vector.tensor_copy(dest, ps)

                proj_serial(
                    [(nb, u) for nb in range(NB) for u in range(2)],
                    lambda g, cb: kvTs[cb][:, 128 * g[0]:128 * (g[0] + 1)],
                    lambda g, cb: WkvTs[cb][:, C + 384 * g[1]:
                                            C + 384 * (g[1] + 1)],
                    v_evac, 128, 384)

            with tc.tile_pool(name="oa", bufs=1) as poa:
                Oall = poa.tile([97, HN], DT16, tag="Oall")
                # Wproj tiles load during P5 (the pool opens after wkv/wq free)
                Wp = []
                for h in range(H):
                    rows = 97 if h == H - 1 else 96
                    t = poa.tile([rows, C], DT16, tag=f"Wp{h}", name=f"Wp{h}")
                    nc.sync.dma_start(t[0:96, :], WpjT[96 * h:96 * (h + 1), :])
                    Wp.append(t)
                nc.sync.dma_start(Wp[H - 1][96:97, :], bias[:])

                with tc.tile_pool(name="pt", bufs=10) as ppt:
                    def emit_S(h, P_of, kb_start=0):
                        for kb in range(kb_start, NB):
                            ps = pmm.tile([128, 1024], F32, tag="mm",
                                          name=f"s{h}_{kb}")
                            for u in range(2):
                                nc.tensor.matmul(
                                    ps[:, 512 * u:512 * (u + 1)],
                                    KT[:, 1024 * h + 128 * kb:
                                       1024 * h + 128 * (kb + 1)],
                                    QT[:, 1024 * h + 512 * u:
                                       1024 * h + 512 * (u + 1)],
                                    start=True, stop=True)
                            nc.scalar.activation(P_of[kb][:], ps[:], AF.Exp)

                    def emit_PV(h, P_of):
                        for u in range(2):
                            po = pso.tile([97, 512], F32, tag="po",
                                          name=f"po{h}_{u}")
                            for kb in range(NB):
                                nc.tensor.matmul(
                                    po[:], V[kb][:, 97 * h:97 * (h + 1)],
                                    P_of[kb][:, 512 * u:512 * (u + 1)],
                                    start=(kb == 0), stop=(kb == NB - 1))
                            nc.vector.tensor_copy(
                                Oall[:, 1024 * h + 512 * u:
                                     1024 * h + 512 * (u + 1)], po[:])

                    ones97f = pn.tile([1, 97], F32, tag="o97f")
                    nc.vector.memset(ones97f[:], 1.0)
                    ones97 = pn.tile([1, 97], DT16, tag="o97")
                    nc.vector.tensor_copy(ones97[:], ones97f[:])

                    def emit_norm(h):
                        """rowsum -> 1/rowsum broadcast to all partitions ->
                        in-place normalize Oall's head-h slice."""
                        sl = slice(1024 * h, 1024 * (h + 1))
                        nc.sync.dma_start(rs_dram[0:1, sl], Oall[96:97, sl])
                        rsh = pn.tile([128, 8], DT16, tag="rs", name=f"rs{h}", bufs=2)
                        nc.sync.dma_start(
                            rsh[:],
                            rs_dram[0:1, sl].rearrange(
                                "p (a b) -> (p a) b", a=128))
                        rih = pn.tile([128, 8], F32, tag="ri", name=f"ri{h}", bufs=2)
                        nc.vector.reciprocal(rih[:], rsh[:])
                        rirh = pn.tile([128, 8], DT16, tag="rir",
                                       name=f"rir{h}", bufs=2)
                        nc.vector.tensor_copy(rirh[:], rih[:])
                        nc.sync.dma_start(
                            ri_dram[0:1, sl].rearrange(
                                "p (a b) -> (p a) b", a=128), rirh[:])
                        bch = pn.tile([97, N], DT16, tag="bc", name=f"bc{h}")
                        nc.sync.dma_start(
                            bch[:], bass.AP(ri_dram.tensor, 1024 * h,
                                            [[0, 97], [1, N]]))
                        nc.vector.tensor_mul(Oall[:, sl], Oall[:, sl], bch[:])

                    def emit_norm_fast_pre(h):
                        """inv = exp(-ln(rowsum)) on ACT (same table set as
                        the softmax Exp); returns the inverse-rowsum row."""
                        sl = slice(1024 * h, 1024 * (h + 1))
                        lnr = pn.tile([1, N], F32, tag="lnx", name=f"lnr{h}")
                        nc.scalar.activation(lnr[:], Oall[96:97, sl], AF.Ln)
                        invt = pn.tile([1, N], DT16, tag="invr",
                                       name=f"invr{h}")
                        nc.scalar.activation(invt[:], lnr[:], AF.Exp,
                                             scale=-1.0)
                        return invt[:]

                    def emit_norm_fast_u(h, u, invr):
                        """Broadcast 1/rowsum via a K=1 ones matmul on PE,
                        multiply the u-half of Oall's head-h slice."""
                        bc_ps = pso.tile([97, 512], F32, tag="po",
                                         name=f"bcps{h}_{u}")
                        nc.tensor.matmul(
                            bc_ps[:], ones97[:],
                            invr[0:1, 512 * u:512 * (u + 1)],
                            start=True, stop=True)
                        ssl = slice(1024 * h + 512 * u,
                                    1024 * h + 512 * (u + 1))
                        nc.vector.tensor_mul(Oall[:, ssl], Oall[:, ssl],
                                             bc_ps[:])

                    # Output projection is split: heads 0-5 accumulate into
                    # f32 SBUF staging (ysbA) while heads 6/7 attention still
                    # runs (fills PE stalls on the ACT norm chain); the tail
                    # only computes heads 6-7, adds the staged partial, and
                    # stores.
                    ysbA = [pn.tile([128, C], F32, tag=f"yA{nb}",
                                    name=f"ysbA{nb}") for nb in range(NB)]

                    def emit_yprojA(nb):
                        for u in range(2):
                            ps = pso.tile([128, 512], F32, tag="po",
                                          name=f"ypA{nb}_{u}")[:, 0:384]
                            for h in range(6):
                                nc.tensor.matmul(
                                    ps,
                                    Oall[0:96, 1024 * h + 128 * nb:
                                         1024 * h + 128 * (nb + 1)],
                                    Wp[h][0:96, 384 * u:384 * (u + 1)],
                                    start=(h == 0), stop=(h == 5))
                            nc.vector.tensor_copy(
                                ysbA[nb][:, 384 * u:384 * (u + 1)], ps)

                    def emit_yprojB(nb, py):
                        ysb = py.tile([128, C], DT16, tag="ysb",
                                      name=f"ysbB{nb}")
                        for u in range(2):
                            ps = pso.tile([128, 512], F32, tag="po",
                                          name=f"ypB{nb}_{u}")[:, 0:384]
                            for i, h in enumerate((6, 7)):
                                rows = 97 if h == H - 1 else 96
                                nc.tensor.matmul(
                                    ps,
                                    Oall[0:rows, 1024 * h + 128 * nb:
                                         1024 * h + 128 * (nb + 1)],
                                    Wp[h][0:rows, 384 * u:384 * (u + 1)],
                                    start=(i == 0), stop=(i == 1))
                            nc.vector.tensor_add(
                                ysb[:, 384 * u:384 * (u + 1)],
                                ysbA[nb][:, 384 * u:384 * (u + 1)], ps)
                        nc.sync.dma_start(
                            y[128 * nb:128 * (nb + 1), :], ysb[:])

                    with tc.tile_pool(name="yout", bufs=2) as py:
                        P_tiles = {}
                        P_tiles[0] = [P0_first] + [
                            ppt.tile([128, N], DT16, tag="pt", name=f"P0_{i}")
                            for i in range(1, NB)]
                        emit_S(0, P_tiles[0], kb_start=1)
                        for h in range(H):
                            if h + 1 < H:
                                P_tiles[h + 1] = [
                                    ppt.tile([128, N], DT16, tag="pt",
                                             name=f"P{h + 1}_{i}")
                                    for i in range(NB)]
                                emit_S(h + 1, P_tiles[h + 1])
                            emit_PV(h, P_tiles.pop(h))
                            if h == 7:
                                for nb in range(4, NB):
                                    emit_yprojA(nb)
                                emit_norm_fast(7)
                            else:
                                emit_norm(h)
                            if h == 6:
                                for nb in range(4):
                                    emit_yprojA(nb)
                        for nb in range(NB):
                            emit_yprojB(nb, py)

    _legalize_waits(nc)
    return nc


def prep_in_maps(x, kv, Wq, Wkv, Wproj, bproj):
    """Host-side prep: transpose + fp16 cast, one in_map per core/batch."""
    x = np.asarray(x, dtype=np.float32)
    kv = np.asarray(kv, dtype=np.float32)
    WqTs = (np.ascontiguousarray(np.asarray(Wq, np.float32).T)
            * np.float32(SCALE)).astype(NP16)
    WkvT = np.ascontiguousarray(np.asarray(Wkv, np.float32).T).astype(NP16)
    WpjT = np.ascontiguousarray(np.asarray(Wproj, np.float32).T).astype(NP16)
    bias_np = np.asarray(bproj, np.float32).reshape(1, C).astype(NP16)
    in_maps = []
    for b in range(B):
        in_maps.append({
            "xT": np.ascontiguousarray(x[b].T).astype(NP16),
            "kvT": np.ascontiguousarray(kv[b].T).astype(NP16),
            "WqT": WqTs,
            "WkvT": WkvT,
            "WpjT": WpjT,
            "bias": bias_np,
        })
    return in_maps


_NC_CACHE = {}


def kernel(x, kv, Wq, Wkv, Wproj, bproj, _trace=False):
    in_maps = prep_in_maps(x, kv, Wq, Wkv, Wproj, bproj)
    if "nc" not in _NC_CACHE:
        _NC_CACHE["nc"] = build_kernel()
    nc = _NC_CACHE["nc"]
    res = run_bass_kernel_spmd(nc, in_maps, core_ids=list(range(B)),
                               trace=_trace)
    out = np.stack([r["y"] for r in res.results]).astype(np.float32)
    if _trace:
        return out, res
    return out
